# revision 28
# baseline (speedup 1.0000x reference)
"""Trainium2 Bass kernel for the ACE vq_codebook problem.

Computation (per reference):
    h0 = relu(x @ enc_w0 + enc_b0)        [B, 2048]
    h1 = relu(h0 @ enc_w1 + enc_b1)       [B, 1024]
    z  = h1 @ enc_w2 + enc_b2             [B, 256]
    d0 = relu(z @ dec_w0 + dec_b0)        [B, 1024]
    d1 = relu(d0 @ dec_w1 + dec_b1)       [B, 2048]
    recon = d1 @ dec_w2 + dec_b2          [B, 1024]
    scores[b,c,k] = <z_b, clus_w[c,k]> + clus_b[c,k]
    conf = -logsumexp(-10*scores, axis=k) [B, 64]
Returns (z, recon, conf).

Strategy: data-parallel over batch across 8 NeuronCores (1024 rows each);
weights replicated. On-chip all MLP activations live transposed
([feature, batch]) so matmul contraction dims sit on SBUF partitions; the
host pre-transposes x and the cluster weight matrix, and transposes
z/recon back after the run. Matmuls run in float32r (full PE rate for
moving dim >= 256, ~1e-4 relative precision).
"""

import os
import sys

import numpy as np

# Persistent XLA/NEFF compile cache: makes repeated invocations cheap.
os.environ.setdefault("JAX_COMPILATION_CACHE_DIR", "/tmp/jax_cc_cache")
os.environ.setdefault("JAX_PERSISTENT_CACHE_MIN_COMPILE_TIME_SECS", "1")
os.environ.setdefault("JAX_PERSISTENT_CACHE_MIN_ENTRY_SIZE_BYTES", "0")

try:
    import concourse.bass as bass  # noqa: F401
except ImportError:
    for p in ("/opt/trn_rl_repo", "/root/.axon_site/_ro/trn_rl_repo"):
        if p not in sys.path:
            sys.path.append(p)

import concourse.bass as bass  # noqa: E402
import concourse.mybir as mybir  # noqa: E402
import concourse.tile as tile  # noqa: E402
from concourse import bacc  # noqa: E402
from concourse.bass_utils import run_bass_kernel_spmd  # noqa: E402

N_CORES = 8
B = 1024  # per-core batch (8192 / 8)
F32 = mybir.dt.float32
F32R = mybir.dt.float32r
AF = mybir.ActivationFunctionType

# (name, Fin, Fout, relu) for the six MLP layers, in execution order.
LAYERS = [
    ("enc_w0", 1024, 2048, True),
    ("enc_w1", 2048, 1024, True),
    ("enc_w2", 1024, 256, False),
    ("dec_w0", 256, 1024, True),
    ("dec_w1", 1024, 2048, True),
    ("dec_w2", 2048, 1024, False),
]
BIAS_OF = {
    "enc_w0": "enc_b0", "enc_w1": "enc_b1", "enc_w2": "enc_b2",
    "dec_w0": "dec_b0", "dec_w1": "dec_b1", "dec_w2": "dec_b2",
}
# SBUF buffer tag per layer output (bufs=1 slots reused across layers).
OUT_TAG = {
    "enc_w0": "bufA",  # h0T  [128,16,B]
    "enc_w1": "bufB",  # h1T  [128, 8,B]  (reuses xT slot)
    "enc_w2": "bufZ",  # zT   [128, 2,B]  (kept for cluster head)
    "dec_w0": "bufB",  # d0T
    "dec_w1": "bufA",  # d1T
    "dec_w2": "bufB",  # reconT
}

_NC_CACHE = {}


def _build(with_clus_bias: bool):
    nc = bacc.Bacc()

    wp = {}
    for name, fin, fout, _ in LAYERS:
        wp[name] = nc.declare_dram_parameter(name, [fin, fout], F32, isOutput=False)
    n_bias_cols = sum(fout // 128 for _, _, fout, _ in LAYERS)
    bias_all = nc.declare_dram_parameter(
        "bias_all", [128, n_bias_cols], F32, isOutput=False
    )
    xT = nc.declare_dram_parameter("xT", [1024, B], F32, isOutput=False)
    cw = nc.declare_dram_parameter("cw", [256, 4096], F32, isOutput=False)
    cb = nc.declare_dram_parameter("cb", [4096], F32, isOutput=False)
    ones = nc.declare_dram_parameter("ones", [1, 128], F32, isOutput=False)
    zT_out = nc.declare_dram_parameter("zT_out", [256, B], F32, isOutput=True)
    reconT_out = nc.declare_dram_parameter("reconT_out", [1024, B], F32, isOutput=True)
    conf_out = nc.declare_dram_parameter("conf_out", [B, 64], F32, isOutput=True)

    with tile.TileContext(nc) as tc:
        with (
            tc.tile_pool(name="acts", bufs=1) as acts,
            tc.tile_pool(name="weights", bufs=6) as wpool,
            tc.tile_pool(name="psum", bufs=8, space="PSUM") as pp,
            tc.tile_pool(name="misc", bufs=1) as misc,
            tc.tile_pool(name="epool", bufs=4) as ep,
        ):
            # Critical-path DMAs first: every dma_start costs ~0.6us of
            # sequencer issue time, so the first weight slab and the input
            # stream must be issued before anything non-critical.
            first_slab = wpool.tile([128, 8, 128], F32R, tag="wslab")
            wv0 = wp["enc_w0"][:, :].rearrange("(kt kp) f -> kp kt f", kp=128)
            nc.sync.dma_start(out=first_slab, in_=wv0[:, :, 0:128].bitcast(F32R))

            # Input activations, transposed: [128, 8 k-tiles, B].
            # Chunked per k-tile so the first matmuls can start as soon as
            # their contraction slice lands instead of after the full 4MB.
            cur = acts.tile([128, 8, B], F32R, tag="bufB")
            xTv = xT[:, :].rearrange("(kt kp) b -> kp kt b", kp=128)
            for half in range(2):
                for ki in range(8):
                    eng = nc.sync if ki % 2 == 0 else nc.gpsimd
                    eng.dma_start(
                        out=cur[:, ki, half * 512:(half + 1) * 512],
                        in_=xTv[:, ki, half * 512:(half + 1) * 512].bitcast(F32R),
                    )

            # Biases (single packed DMA, not needed until the first
            # activation ~25us in).
            bias_t = misc.tile([128, n_bias_cols], F32, tag="bias_all")
            nc.sync.dma_start(out=bias_t, in_=bias_all[:, :])
            b_sb = {}
            off = 0
            for name, fin, fout, _ in LAYERS:
                b_sb[BIAS_OF[name]] = bias_t[:, off:off + fout // 128]
                off += fout // 128

            def cluster_head(zT_t):
                # scores in natural [B, ck] layout:
                # scores[b, ck] = sum_d z[b, d] * cw[d, ck] (+ cb[ck])
                S_all = ep.tile([128, 8, 64], F32, tag="S")  # [Bp, btile, c]
                cwv = cw[:, :].rearrange("(kt kp) f -> kp kt f", kp=128)
                for ch in range(8):  # 512 ck columns per chunk
                    cslab = wpool.tile([128, 2, 512], F32R, tag="wslab")
                    nc.sync.dma_start(
                        out=cslab,
                        in_=cwv[:, :, ch * 512:(ch + 1) * 512].bitcast(F32R),
                    )
                    if with_clus_bias:
                        cb_sb = wpool.tile([1, 512], F32R, tag="cb")
                        nc.sync.dma_start(
                            out=cb_sb,
                            in_=cb[ch * 512:(ch + 1) * 512]
                            .rearrange("(o f) -> o f", o=1)
                            .bitcast(F32R),
                        )
                    for bt8 in range(8):  # batch tiles of 128
                        ps = pp.tile([128, 512], F32, tag="ps")
                        nc.tensor.matmul(
                            ps, zT_t[:, 0, bt8 * 128:(bt8 + 1) * 128],
                            cslab[:, 0, :], start=True, stop=False,
                        )
                        nc.tensor.matmul(
                            ps, zT_t[:, 1, bt8 * 128:(bt8 + 1) * 128],
                            cslab[:, 1, :], start=False,
                            stop=not with_clus_bias,
                        )
                        if with_clus_bias:
                            nc.tensor.matmul(
                                ps, ones_t[0:1, :], cb_sb[0:1, :],
                                start=False, stop=True,
                            )
                        e_t = ep.tile([128, 512], F32, tag="e")
                        nc.scalar.activation(e_t, ps, AF.Exp, scale=-10.0)
                        nc.vector.reduce_sum(
                            S_all[:, bt8, ch * 8:(ch + 1) * 8],
                            e_t[:, :].rearrange("p (c k) -> p c k", k=64),
                            axis=mybir.AxisListType.X,
                        )
                return S_all

            if with_clus_bias:
                ones_t = misc.tile([1, 128], F32R, tag="ones")
                nc.sync.dma_start(out=ones_t, in_=ones[:, :].bitcast(F32R))

            zT_t = None
            for name, fin, fout, relu in LAYERS:
                nk = fin // 128
                outT = acts.tile([128, fout // 128, B], F32R, tag=OUT_TAG[name])
                wv = wp[name][:, :].rearrange("(kt kp) f -> kp kt f", kp=128)
                bias = b_sb[BIAS_OF[name]]
                # narrow slabs keep the slab pool small (deep prefetch)
                # and shorten the HBM-bound startup critical path
                CH = 128 if fin >= 2048 or name == "enc_w0" else 256
                for ch in range(fout // CH):
                    if name == "enc_w0" and ch == 0:
                        slab = first_slab
                    else:
                        slab = wpool.tile([128, nk, CH], F32R, tag="wslab")
                        nc.sync.dma_start(
                            out=slab,
                            in_=wv[:, :, ch * CH:(ch + 1) * CH].bitcast(F32R),
                        )
                    for fl in range(CH // 128):
                        fo = ch * (CH // 128) + fl
                        for bt in range(B // 512):
                            ps = pp.tile([128, 512], F32, tag="ps")
                            for ki in range(nk):
                                nc.tensor.matmul(
                                    ps,
                                    slab[:, ki, fl * 128:(fl + 1) * 128],
                                    cur[:, ki, bt * 512:(bt + 1) * 512],
                                    start=(ki == 0),
                                    stop=(ki == nk - 1),
                                )
                            dst = outT[:, fo, bt * 512:(bt + 1) * 512]
                            if relu:
                                nc.scalar.activation(
                                    dst, ps, AF.Relu,
                                    bias=bias[:, fo:fo + 1], scale=1.0,
                                )
                            else:
                                nc.vector.tensor_scalar_add(
                                    dst, ps, bias[:, fo:fo + 1]
                                )
                            if name == "dec_w2":
                                # stream recon out per half-batch so the
                                # final DMA after the last matmul is small
                                nc.gpsimd.dma_start(
                                    out=reconT_out[
                                        fo * 128:(fo + 1) * 128,
                                        bt * 512:(bt + 1) * 512,
                                    ].bitcast(F32R),
                                    in_=outT[:, fo, bt * 512:(bt + 1) * 512],
                                )
                cur = outT
                if name == "enc_w2":
                    zT_t = outT
                    nc.gpsimd.dma_start(
                        out=zT_out[:, :]
                        .rearrange("(t p) b -> p t b", p=128)
                        .bitcast(F32R),
                        in_=zT_t,
                    )
                    S_all = cluster_head(zT_t)

            # conf finalization deferred past the decoder: the Ln
            # activation-table load (1.3us) then lands in the epilogue
            # instead of stalling the mid-decoder ACT->psum->PE pipeline.
            lnS = ep.tile([128, 8, 64], F32, tag="lnS")
            nc.scalar.activation(lnS, S_all, AF.Ln)
            conf_t = ep.tile([128, 8, 64], F32, tag="conf")
            nc.vector.tensor_scalar_mul(conf_t, lnS, -1.0)
            nc.gpsimd.dma_start(
                out=conf_out[:, :].rearrange("(t p) c -> p t c", p=128),
                in_=conf_t,
            )

    nc.finalize()
    return nc


def _get_nc(with_clus_bias: bool):
    key = bool(with_clus_bias)
    if key not in _NC_CACHE:
        _NC_CACHE[key] = _build(key)
    return _NC_CACHE[key]


def _run(inputs, trace=False):
    f = lambda a: np.ascontiguousarray(np.asarray(a, dtype=np.float32))
    x = f(inputs["x"])  # [8192, 1024]
    clus_w = f(inputs["clus_w"])  # [64, 64, 256]
    clus_b = f(inputs["clus_b"])  # [64, 64]

    with_clus_bias = bool(np.any(clus_b != 0.0))
    nc = _get_nc(with_clus_bias)

    shared = {}
    cols = []
    for name, fin, fout, _ in LAYERS:
        shared[name] = f(inputs[name])
        cols.append(f(inputs[BIAS_OF[name]]).reshape(-1, 128).T)
    shared["bias_all"] = np.ascontiguousarray(np.concatenate(cols, axis=1))
    shared["cw"] = np.ascontiguousarray(clus_w.reshape(4096, 256).T)
    shared["cb"] = clus_b.reshape(4096)
    shared["ones"] = np.ones((1, 128), np.float32)

    in_maps = []
    for c in range(N_CORES):
        m = dict(shared)
        m["xT"] = np.ascontiguousarray(x[c * B:(c + 1) * B].T)
        in_maps.append(m)

    res = run_bass_kernel_spmd(
        nc, in_maps, core_ids=list(range(N_CORES)), trace=trace
    )

    z = np.empty((N_CORES * B, 256), np.float32)
    recon = np.empty((N_CORES * B, 1024), np.float32)
    conf = np.empty((N_CORES * B, 64), np.float32)
    for c in range(N_CORES):
        r = res.results[c]
        z[c * B:(c + 1) * B] = r["zT_out"].T
        recon[c * B:(c + 1) * B] = r["reconT_out"].T
        conf[c * B:(c + 1) * B] = r["conf_out"]
    return (z, recon, conf), res


def kernel(**inputs):
    (z, recon, conf), _ = _run(inputs, trace=False)
    return z, recon, conf


# revision 30
# speedup vs baseline: 1.1892x; 1.1892x over previous
"""Trainium2 Bass kernel for the ACE vq_codebook problem.

Computation (per reference):
    h0 = relu(x @ enc_w0 + enc_b0)        [B, 2048]
    h1 = relu(h0 @ enc_w1 + enc_b1)       [B, 1024]
    z  = h1 @ enc_w2 + enc_b2             [B, 256]
    d0 = relu(z @ dec_w0 + dec_b0)        [B, 1024]
    d1 = relu(d0 @ dec_w1 + dec_b1)       [B, 2048]
    recon = d1 @ dec_w2 + dec_b2          [B, 1024]
    scores[b,c,k] = <z_b, clus_w[c,k]> + clus_b[c,k]
    conf = -logsumexp(-10*scores, axis=k) [B, 64]
Returns (z, recon, conf).

Strategy: data-parallel over batch across 8 NeuronCores (1024 rows each);
weights replicated. On-chip all MLP activations live transposed
([feature, batch]) so matmul contraction dims sit on SBUF partitions; the
host pre-transposes x and the cluster weight matrix, and transposes
z/recon back after the run. Matmuls run in float32r (full PE rate for
moving dim >= 256, ~1e-4 relative precision).
"""

import os
import sys

import numpy as np

# Persistent XLA/NEFF compile cache: makes repeated invocations cheap.
os.environ.setdefault("JAX_COMPILATION_CACHE_DIR", "/tmp/jax_cc_cache")
os.environ.setdefault("JAX_PERSISTENT_CACHE_MIN_COMPILE_TIME_SECS", "1")
os.environ.setdefault("JAX_PERSISTENT_CACHE_MIN_ENTRY_SIZE_BYTES", "0")

try:
    import concourse.bass as bass  # noqa: F401
except ImportError:
    for p in ("/opt/trn_rl_repo", "/root/.axon_site/_ro/trn_rl_repo"):
        if p not in sys.path:
            sys.path.append(p)

import concourse.bass as bass  # noqa: E402
import concourse.mybir as mybir  # noqa: E402
import concourse.tile as tile  # noqa: E402
from concourse import bacc  # noqa: E402
from concourse.bass_utils import run_bass_kernel_spmd  # noqa: E402

N_CORES = 8
B = 1024  # per-core batch (8192 / 8)
F32 = mybir.dt.float32
F32R = mybir.dt.float32r
AF = mybir.ActivationFunctionType

# (name, Fin, Fout, relu) for the six MLP layers, in execution order.
LAYERS = [
    ("enc_w0", 1024, 2048, True),
    ("enc_w1", 2048, 1024, True),
    ("enc_w2", 1024, 256, False),
    ("dec_w0", 256, 1024, True),
    ("dec_w1", 1024, 2048, True),
    ("dec_w2", 2048, 1024, False),
]
BIAS_OF = {
    "enc_w0": "enc_b0", "enc_w1": "enc_b1", "enc_w2": "enc_b2",
    "dec_w0": "dec_b0", "dec_w1": "dec_b1", "dec_w2": "dec_b2",
}
# SBUF buffer tag per layer output (bufs=1 slots reused across layers).
OUT_TAG = {
    "enc_w0": "bufA",  # h0T  [128,16,B]
    "enc_w1": "bufB",  # h1T  [128, 8,B]  (reuses xT slot)
    "enc_w2": "bufZ",  # zT   [128, 2,B]  (kept for cluster head)
    "dec_w0": "bufB",  # d0T
    "dec_w1": "bufA",  # d1T
    "dec_w2": "bufB",  # reconT
}

_NC_CACHE = {}


def _build(with_clus_bias: bool):
    nc = bacc.Bacc()

    wp = {}
    for name, fin, fout, _ in LAYERS:
        wp[name] = nc.declare_dram_parameter(name, [fin, fout], F32, isOutput=False)
    n_bias_cols = sum(fout // 128 for _, _, fout, _ in LAYERS)
    bias_all = nc.declare_dram_parameter(
        "bias_all", [128, n_bias_cols], F32, isOutput=False
    )
    xT = nc.declare_dram_parameter("xT", [1024, B], F32, isOutput=False)
    cw = nc.declare_dram_parameter("cw", [256, 4096], F32, isOutput=False)
    cb = nc.declare_dram_parameter("cb", [4096], F32, isOutput=False)
    ones = nc.declare_dram_parameter("ones", [1, 128], F32, isOutput=False)
    zT_out = nc.declare_dram_parameter("zT_out", [256, B], F32, isOutput=True)
    reconT_out = nc.declare_dram_parameter("reconT_out", [1024, B], F32, isOutput=True)
    conf_out = nc.declare_dram_parameter("conf_out", [B, 64], F32, isOutput=True)

    with tile.TileContext(nc) as tc:
        with (
            tc.tile_pool(name="acts", bufs=1) as acts,
            tc.tile_pool(name="weights", bufs=6) as wpool,
            tc.tile_pool(name="psum", bufs=8, space="PSUM") as pp,
            tc.tile_pool(name="misc", bufs=1) as misc,
            tc.tile_pool(name="epool", bufs=4) as ep,
        ):
            # Critical-path DMAs first: every dma_start costs ~0.6us of
            # sequencer issue time, so the first weight slab and the input
            # stream must be issued before anything non-critical.
            first_slab = wpool.tile([128, 8, 128], F32R, tag="wslab")
            wv0 = wp["enc_w0"][:, :].rearrange("(kt kp) f -> kp kt f", kp=128)
            nc.sync.dma_start(out=first_slab, in_=wv0[:, :, 0:128].bitcast(F32R))

            # Input activations, transposed: [128, 8 k-tiles, B].
            # Chunked per k-tile so the first matmuls can start as soon as
            # their contraction slice lands instead of after the full 4MB.
            cur = acts.tile([128, 8, B], F32R, tag="bufB")
            xTv = xT[:, :].rearrange("(kt kp) b -> kp kt b", kp=128)
            for half in range(2):
                for ki in range(8):
                    eng = (nc.sync, nc.gpsimd, nc.scalar)[ki % 3]
                    eng.dma_start(
                        out=cur[:, ki, half * 512:(half + 1) * 512],
                        in_=xTv[:, ki, half * 512:(half + 1) * 512].bitcast(F32R),
                    )

            # Biases (single packed DMA, not needed until the first
            # activation ~25us in).
            bias_t = misc.tile([128, n_bias_cols], F32, tag="bias_all")
            nc.sync.dma_start(out=bias_t, in_=bias_all[:, :])
            b_sb = {}
            off = 0
            for name, fin, fout, _ in LAYERS:
                b_sb[BIAS_OF[name]] = bias_t[:, off:off + fout // 128]
                off += fout // 128

            def cluster_head(zT_t):
                # scores in natural [B, ck] layout:
                # scores[b, ck] = sum_d z[b, d] * cw[d, ck] (+ cb[ck])
                S_all = ep.tile([128, 8, 64], F32, tag="S")  # [Bp, btile, c]
                cwv = cw[:, :].rearrange("(kt kp) f -> kp kt f", kp=128)
                for ch in range(8):  # 512 ck columns per chunk
                    cslab = wpool.tile([128, 2, 512], F32R, tag="wslab")
                    nc.sync.dma_start(
                        out=cslab,
                        in_=cwv[:, :, ch * 512:(ch + 1) * 512].bitcast(F32R),
                    )
                    if with_clus_bias:
                        cb_sb = wpool.tile([1, 512], F32R, tag="cb")
                        nc.sync.dma_start(
                            out=cb_sb,
                            in_=cb[ch * 512:(ch + 1) * 512]
                            .rearrange("(o f) -> o f", o=1)
                            .bitcast(F32R),
                        )
                    for bt8 in range(8):  # batch tiles of 128
                        ps = pp.tile([128, 512], F32, tag="ps")
                        nc.tensor.matmul(
                            ps, zT_t[:, 0, bt8 * 128:(bt8 + 1) * 128],
                            cslab[:, 0, :], start=True, stop=False,
                        )
                        nc.tensor.matmul(
                            ps, zT_t[:, 1, bt8 * 128:(bt8 + 1) * 128],
                            cslab[:, 1, :], start=False,
                            stop=not with_clus_bias,
                        )
                        if with_clus_bias:
                            nc.tensor.matmul(
                                ps, ones_t[0:1, :], cb_sb[0:1, :],
                                start=False, stop=True,
                            )
                        e_t = ep.tile([128, 512], F32, tag="e")
                        nc.scalar.activation(e_t, ps, AF.Exp, scale=-10.0)
                        nc.vector.reduce_sum(
                            S_all[:, bt8, ch * 8:(ch + 1) * 8],
                            e_t[:, :].rearrange("p (c k) -> p c k", k=64),
                            axis=mybir.AxisListType.X,
                        )
                return S_all

            if with_clus_bias:
                ones_t = misc.tile([1, 128], F32R, tag="ones")
                nc.sync.dma_start(out=ones_t, in_=ones[:, :].bitcast(F32R))

            zT_t = None
            for name, fin, fout, relu in LAYERS:
                nk = fin // 128
                outT = acts.tile([128, fout // 128, B], F32R, tag=OUT_TAG[name])
                wv = wp[name][:, :].rearrange("(kt kp) f -> kp kt f", kp=128)
                bias = b_sb[BIAS_OF[name]]
                # narrow slabs keep the slab pool small (deep prefetch)
                # and shorten the HBM-bound startup critical path
                CH = 128 if fin >= 2048 or name == "enc_w0" else 256
                for ch in range(fout // CH):
                    if name == "enc_w0" and ch == 0:
                        slab = first_slab
                    else:
                        slab = wpool.tile([128, nk, CH], F32R, tag="wslab")
                        nc.sync.dma_start(
                            out=slab,
                            in_=wv[:, :, ch * CH:(ch + 1) * CH].bitcast(F32R),
                        )
                    for fl in range(CH // 128):
                        fo = ch * (CH // 128) + fl
                        for bt in range(B // 512):
                            ps = pp.tile([128, 512], F32, tag="ps")
                            for ki in range(nk):
                                nc.tensor.matmul(
                                    ps,
                                    slab[:, ki, fl * 128:(fl + 1) * 128],
                                    cur[:, ki, bt * 512:(bt + 1) * 512],
                                    start=(ki == 0),
                                    stop=(ki == nk - 1),
                                )
                            dst = outT[:, fo, bt * 512:(bt + 1) * 512]
                            if relu:
                                nc.scalar.activation(
                                    dst, ps, AF.Relu,
                                    bias=bias[:, fo:fo + 1], scale=1.0,
                                )
                            else:
                                nc.vector.tensor_scalar_add(
                                    dst, ps, bias[:, fo:fo + 1]
                                )
                            if name == "dec_w2":
                                # stream recon out per half-batch so the
                                # final DMA after the last matmul is small
                                nc.gpsimd.dma_start(
                                    out=reconT_out[
                                        fo * 128:(fo + 1) * 128,
                                        bt * 512:(bt + 1) * 512,
                                    ].bitcast(F32R),
                                    in_=outT[:, fo, bt * 512:(bt + 1) * 512],
                                )
                cur = outT
                if name == "dec_w1":
                    # conf finalization emitted here (not after the loop, not
                    # right after the cluster head): the Ln's activation-table
                    # load and the Ln itself land mid-stream where psum depth
                    # absorbs the Scalar-engine hiccup, and the conf DMA
                    # completes under L6 -- the kernel tail stays pure recon.
                    lnS = ep.tile([128, 8, 64], F32, tag="lnS")
                    nc.scalar.activation(lnS, S_all, AF.Ln)
                    conf_t = ep.tile([128, 8, 64], F32, tag="conf")
                    nc.vector.tensor_scalar_mul(conf_t, lnS, -1.0)
                    nc.gpsimd.dma_start(
                        out=conf_out[:, :].rearrange("(t p) c -> p t c", p=128),
                        in_=conf_t,
                    )
                if name == "enc_w2":
                    zT_t = outT
                    nc.gpsimd.dma_start(
                        out=zT_out[:, :]
                        .rearrange("(t p) b -> p t b", p=128)
                        .bitcast(F32R),
                        in_=zT_t,
                    )
                    S_all = cluster_head(zT_t)

    nc.finalize()
    return nc


def _get_nc(with_clus_bias: bool):
    key = bool(with_clus_bias)
    if key not in _NC_CACHE:
        _NC_CACHE[key] = _build(key)
    return _NC_CACHE[key]


def _run(inputs, trace=False):
    f = lambda a: np.ascontiguousarray(np.asarray(a, dtype=np.float32))
    x = f(inputs["x"])  # [8192, 1024]
    clus_w = f(inputs["clus_w"])  # [64, 64, 256]
    clus_b = f(inputs["clus_b"])  # [64, 64]

    with_clus_bias = bool(np.any(clus_b != 0.0))
    nc = _get_nc(with_clus_bias)

    shared = {}
    cols = []
    for name, fin, fout, _ in LAYERS:
        shared[name] = f(inputs[name])
        cols.append(f(inputs[BIAS_OF[name]]).reshape(-1, 128).T)
    shared["bias_all"] = np.ascontiguousarray(np.concatenate(cols, axis=1))
    shared["cw"] = np.ascontiguousarray(clus_w.reshape(4096, 256).T)
    shared["cb"] = clus_b.reshape(4096)
    shared["ones"] = np.ones((1, 128), np.float32)

    in_maps = []
    for c in range(N_CORES):
        m = dict(shared)
        m["xT"] = np.ascontiguousarray(x[c * B:(c + 1) * B].T)
        in_maps.append(m)

    res = run_bass_kernel_spmd(
        nc, in_maps, core_ids=list(range(N_CORES)), trace=trace
    )

    z = np.empty((N_CORES * B, 256), np.float32)
    recon = np.empty((N_CORES * B, 1024), np.float32)
    conf = np.empty((N_CORES * B, 64), np.float32)
    for c in range(N_CORES):
        r = res.results[c]
        z[c * B:(c + 1) * B] = r["zT_out"].T
        recon[c * B:(c + 1) * B] = r["reconT_out"].T
        conf[c * B:(c + 1) * B] = r["conf_out"]
    return (z, recon, conf), res


def kernel(**inputs):
    (z, recon, conf), _ = _run(inputs, trace=False)
    return z, recon, conf
